# revision 1
# baseline (speedup 1.0000x reference)
"""DCNv2 (modulated deformable conv) forward on 8 Trainium2 NeuronCores.

Problem: input [4,64,96,96], offset [4,18,96,96], mask [4,9,96,96],
weight [64,64,3,3], bias [64] -> out [4,64,96,96]. STRIDE=1, PAD=1, DIL=1,
deformable groups G=1.

Sharding: data-parallel over (batch, H-half): core = b*2 + h handles output
rows [48h, 48h+48) of batch b; weight/bias replicated.

The kernel is bound end-to-end by stock SWDGE indirect-DMA issue: each
gather instruction carries exactly 128 descriptors (1 row-index per dest
partition is the only indirect form this firmware supports; multi-index
InstDMACopy silently drops indices beyond the first per partition, and the
Ant dma_gather/ap_gather ucode crashes the device) and costs a fixed
~994ns ucode generation + ~310ns issue gap regardless of bytes.  36 tiles
x 9 taps = 324 instructions => ~455us floor.  Everything else overlaps
under that stream.  vs the 600us baseline (measured ~486.8us):
  * bf16 quad-packed image (256B rows, 512B gathered quads) halves ring
    traffic; src-row=dest/2 layout keeps the fast 1089ns ucode path.
  * Layout-only host prep: quad-packed pad tensors (per-core-base via the
    basev input; uniform PAD_ROWS so one compiled module serves all
    cores), pixel-major offset/mask, base positions, rearranged weights.
    The offset-dependent math (floor, fractions, validity masks,
    bilinear*mask weights) stays on device.
  * Index math split y+x-combined and idx-first (a3a), weights after
    (a3b), with the leading tiles' gathers emitted between them so the
    gather stream starts ~13us in (chunk 0 = one tile with its own
    27-column offset load).
  * The per-tile bilinear weights are expanded (k,l,v)->(k,l,v,c) on the
    otherwise-idle Scalar engine so the DVE multiply is a contiguous
    bf16 2-els/cycle op; DVE SBUF traffic otherwise stretches the SWDGE
    descriptor-generation ucode (~47us of per-tile stalls without this).
  * Gathers lead compute by LEAD tiles; bias-add on Scalar (Identity).
    The final tile gathers into nine per-tap tiles so its drain pipeline
    overlaps its own gather window (Tile deps are tile-granular).
  * _host_prep asserts max|offset| < 6, which keeps every in-bounds
    sample of a tile-half inside its pad tensor (out-of-image samples
    carry zero bilinear weight, so their clamped reads are harmless).

Per-core algorithm:
  pad*[r] = [pixel(base+r-97) | pixel(base+r-1)] (bf16, zero edge pads)
  so ONE contiguous 512B read at local row r = 96*fy+fx+97-base returns
  all four bilinear neighbors [v00, v10, v01, v11] of a sample.
  Per (tile, tap): one indirect DMA (128 rows) -> DVE applies
  bilinear*mask weights -> quad-reduce adds -> samp [128px, 9*64] bf16 ->
  PE transposes (chunks 0,128,256,384,448 with dedup in the weight
  layout) -> fp32 matmul with rearranged weights -> bias -> HWDGE store.
"""

import os
import sys
import types
import numpy as np

for _p in ("/opt/trn_rl_repo",):
    if _p not in sys.path and os.path.isdir(_p):
        sys.path.append(_p)

# Start from a clean core state: an accumulated-state regime was observed
# to inflate exec time ~20% on identical code; a reset clears it.
os.environ.setdefault("NEURON_RT_RESET_CORES", "1")

try:
    import antenv.axon_hooks  # noqa: F401
except ImportError:
    _hookmod = types.ModuleType("antenv.axon_hooks")
    _hookmod._hook = None
    _hookmod.set_axon_ntff_profile_hook = lambda h: setattr(_hookmod, "_hook", h)
    _hookmod.get_axon_ntff_profile_hook = lambda: _hookmod._hook
    sys.modules["antenv.axon_hooks"] = _hookmod

B, C, H, W = 4, 64, 96, 96
K = 9
Co = 64
HW = H * W                  # 9216
N_CORES = 8
HHALF = 48
NPIX = HHALF * W            # 4608 output pixels per core
NT = NPIX // 128            # 36 tiles
NTH = NT // 2               # 18 tiles per pad half
CHUNK_OFFS = (0, 128, 256, 384, 448)  # samp free-dim transpose chunks

PAD_ROWS = 4000             # rows per pad-half tensor (per-core global base
                            # varies and is passed via the basev input)

_CACHE = {}


def _build_module():
    from contextlib import ExitStack

    import concourse.bass as bass
    import concourse.tile as tile
    from concourse import bacc, mybir
    from concourse.bass_interp import get_hw_module
    from concourse.masks import make_identity

    f32 = mybir.dt.float32
    bf16 = mybir.dt.bfloat16
    i32 = mybir.dt.int32
    Alu = mybir.AluOpType
    Act = mybir.ActivationFunctionType

    nc = bacc.Bacc("TRN2", target_bir_lowering=False, debug=False,
                   enable_asserts=False, num_devices=N_CORES,
                   dynamic_dma_scratch_size=32768)

    omT_ap = nc.dram_tensor("omT", [128, NT * 27], f32, kind="ExternalInput").ap()
    byx_ap = nc.dram_tensor("byx", [128, NT * K * 2], f32, kind="ExternalInput").ap()
    w2_ap = nc.dram_tensor("w2", [5 * 128, Co], f32, kind="ExternalInput").ap()
    bias_ap = nc.dram_tensor("biasv", [Co, 1], f32, kind="ExternalInput").ap()
    padT_ap = nc.dram_tensor("padtop", [PAD_ROWS, 2 * C], bf16,
                             kind="ExternalInput").ap()
    padB_ap = nc.dram_tensor("padbot", [PAD_ROWS, 2 * C], bf16,
                             kind="ExternalInput").ap()
    basev_ap = nc.dram_tensor("basev", [128, 2], f32, kind="ExternalInput").ap()
    out_ap = nc.dram_tensor("out", [Co, NPIX], f32, kind="ExternalOutput").ap()

    with tile.TileContext(nc) as tc:
        with ExitStack() as ctx:
            cpool = ctx.enter_context(tc.tile_pool(name="consts", bufs=1))
            apool = ctx.enter_context(tc.tile_pool(name="phase_a", bufs=1))
            prep = ctx.enter_context(tc.tile_pool(name="prep", bufs=1))
            tb_ps = ctx.enter_context(tc.tile_pool(name="trB_ps", bufs=4, space="PSUM"))
            opsum = ctx.enter_context(tc.tile_pool(name="opsum", bufs=2, space="PSUM"))

            V = nc.vector

            # ---- loads that gate the idx chain.  chunk 0 covers one tile
            # so its offset load + index chain are minimal ----
            CHUNKS = ((0, 1, 0), (1, 4, 0), (4, 11, 0), (11, 18, 0),
                      (18, 27, 1), (27, 36, 1))
            omT = [prep.tile([128, (t1 - t0) * 27], f32, name=f"omTc{ci}",
                             tag=f"omTc{ci}")
                   for ci, (t0, t1, h) in enumerate(CHUNKS)]
            nc.sync.dma_start(out=omT[0][:], in_=omT_ap[:, 0:27])
            byx_sb = cpool.tile([128, NT * K * 2], f32)
            nc.sync.dma_start(out=byx_sb[:], in_=byx_ap)
            basev_sb = cpool.tile([128, 2], f32)
            nc.sync.dma_start(out=basev_sb[:], in_=basev_ap)
            for ci, (t0, t1, h) in enumerate(CHUNKS):
                if ci > 0:
                    nc.sync.dma_start(out=omT[ci][:],
                                      in_=omT_ap[:, t0 * 27:t1 * 27])

            ident = cpool.tile([128, 128], f32)
            make_identity(nc, ident[:])
            identb = cpool.tile([128, 128], bf16)
            V.tensor_copy(out=identb[:], in_=ident[:])

            # ---- phase A3 per chunk: idx first, then bilinear weights ----
            idxi = [apool.tile([128, (t1 - t0) * K], i32, name=f"idxi{ci}",
                               tag=f"idxi{ci}")
                    for ci, (t0, t1, h) in enumerate(CHUNKS)]
            wq = [apool.tile([128, (t1 - t0) * K * 4], bf16, name=f"wq{ci}",
                             tag=f"wq{ci}")
                  for ci, (t0, t1, h) in enumerate(CHUNKS)]

            tmp = {}

            def a3a(ci):
                """Gather-index chain: pypx -> floor -> idx (gates gathers).
                y and x are processed together: omT channels 0..17 are
                (dy0,dx0,dy1,dx1,...) and byx matches that (k,s) order."""
                t0, t1, h = CHUNKS[ci]
                nt = t1 - t0
                omT3 = omT[ci][:].rearrange("p (t c) -> p t c", t=nt)
                byx18 = byx_sb[:].rearrange("p (t x) -> p t x", t=NT)[
                    :, t0:t1, :]
                pypx = prep.tile([128, nt * 18], f32, name=f"pypx{ci}",
                                 tag=f"pypx{ci}")
                pypxv = pypx[:].rearrange("p (t x) -> p t x", t=nt)
                f = prep.tile([128, nt * 18], f32, name=f"f{ci}", tag=f"f{ci}")
                fv = f[:].rearrange("p (t x) -> p t x", t=nt)
                w = prep.tile([128, nt * 18], f32, name=f"w{ci}", tag=f"w{ci}")
                wv = w[:].rearrange("p (t x) -> p t x", t=nt)
                ta = prep.tile([128, nt * 18], f32, name=f"tca{ci}", tag=f"tca{ci}")
                tav = ta[:].rearrange("p (t x) -> p t x", t=nt)
                tb = prep.tile([128, nt * 18], f32, name=f"tcb{ci}", tag=f"tcb{ci}")
                tbv = tb[:].rearrange("p (t x) -> p t x", t=nt)
                ti = prep.tile([128, nt * 18], i32, name=f"ti{ci}", tag=f"ti{ci}")
                tiv = ti[:].rearrange("p (t x) -> p t x", t=nt)
                tmp[("f", ci)] = f
                tmp[("w", ci)] = w

                # pypx = d + base; floor via cast-roundtrip (any rounding mode)
                V.tensor_tensor(out=pypxv, in0=omT3[:, :, 0:18], in1=byx18,
                                op=Alu.add)
                V.tensor_copy(out=tiv, in_=pypxv)
                V.tensor_copy(out=tav, in_=tiv)
                V.tensor_tensor(out=tbv, in0=tav, in1=pypxv, op=Alu.is_gt)
                V.tensor_tensor(out=fv, in0=tav, in1=tbv, op=Alu.subtract)

                # idx = clamp(96*fy + fx + (97 - base[core,h]), 0, PAD_ROWS-2)
                f3 = f[:].rearrange("p (t k s) -> p t k s", t=nt, k=K)
                idxf = prep.tile([128, nt * K], f32, name=f"idxf{ci}",
                                 tag=f"idxf{ci}")
                idxfv = idxf[:].rearrange("p (t k) -> p t k", t=nt)
                V.scalar_tensor_tensor(out=idxfv, in0=f3[:, :, :, 0],
                                       scalar=96.0, in1=f3[:, :, :, 1],
                                       op0=Alu.mult, op1=Alu.add)
                V.tensor_scalar(out=idxf[:], in0=idxf[:],
                                scalar1=basev_sb[:, h:h + 1],
                                scalar2=None, op0=Alu.add)
                V.tensor_scalar(out=idxf[:], in0=idxf[:], scalar1=0.0,
                                scalar2=float(PAD_ROWS - 2),
                                op0=Alu.max, op1=Alu.min)
                V.tensor_copy(out=idxi[ci][:], in_=idxf[:])

                # fractional parts (wy, wx interleaved)
                V.tensor_tensor(out=wv, in0=pypxv, in1=fv, op=Alu.subtract)

            def a3b(ci):
                """Bilinear*mask weights (consumed only after gathers land)."""
                t0, t1, h = CHUNKS[ci]
                nt = t1 - t0
                omT3 = omT[ci][:].rearrange("p (t c) -> p t c", t=nt)
                mv = omT3[:, :, 18:27]
                f = tmp[("f", ci)]
                w = tmp[("w", ci)]
                f3 = f[:].rearrange("p (t k s) -> p t k s", t=nt, k=K)
                w3 = w[:].rearrange("p (t k s) -> p t k s", t=nt, k=K)
                fyv, fxv = f3[:, :, :, 0], f3[:, :, :, 1]
                wyv, wxv = w3[:, :, :, 0], w3[:, :, :, 1]

                def t3(name):
                    t = prep.tile([128, nt * K], f32, name=f"{name}{ci}",
                                  tag=f"{name}{ci}")
                    return t, t[:].rearrange("p (t k) -> p t k", t=nt)

                ta, tav = t3("t9a")
                tb, tbv = t3("t9b")
                vm0, vm0v = t3("vm0")
                vm1, vm1v = t3("vm1")
                vc0, vc0v = t3("vc0")
                vc1, vc1v = t3("vc1")
                cA, cAv = t3("cA")
                cB, cBv = t3("cB")
                # row validity (* mask) and column validity
                V.tensor_scalar(out=tav, in0=fyv, scalar1=0.0, scalar2=None, op0=Alu.is_ge)
                V.tensor_scalar(out=tbv, in0=fyv, scalar1=95.0, scalar2=None, op0=Alu.is_le)
                V.tensor_tensor(out=vm0v, in0=tav, in1=tbv, op=Alu.mult)
                V.tensor_tensor(out=vm0v, in0=vm0v, in1=mv, op=Alu.mult)
                V.tensor_scalar(out=tav, in0=fyv, scalar1=-1.0, scalar2=None, op0=Alu.is_ge)
                V.tensor_scalar(out=tbv, in0=fyv, scalar1=94.0, scalar2=None, op0=Alu.is_le)
                V.tensor_tensor(out=vm1v, in0=tav, in1=tbv, op=Alu.mult)
                V.tensor_tensor(out=vm1v, in0=vm1v, in1=mv, op=Alu.mult)
                V.tensor_scalar(out=tav, in0=fxv, scalar1=0.0, scalar2=None, op0=Alu.is_ge)
                V.tensor_scalar(out=tbv, in0=fxv, scalar1=95.0, scalar2=None, op0=Alu.is_le)
                V.tensor_tensor(out=vc0v, in0=tav, in1=tbv, op=Alu.mult)
                V.tensor_scalar(out=tav, in0=fxv, scalar1=-1.0, scalar2=None, op0=Alu.is_ge)
                V.tensor_scalar(out=tbv, in0=fxv, scalar1=94.0, scalar2=None, op0=Alu.is_le)
                V.tensor_tensor(out=vc1v, in0=tav, in1=tbv, op=Alu.mult)

                # bilinear coefficients: cy0/cy1 (carry mask), cx0/cx1
                # (1 - w) computed on DVE via dual-op tensor_scalar
                V.tensor_scalar(out=tav, in0=wyv, scalar1=-1.0, scalar2=1.0,
                                op0=Alu.mult, op1=Alu.add)
                V.tensor_tensor(out=cAv, in0=tav, in1=vm0v, op=Alu.mult)   # cy0
                V.tensor_tensor(out=cBv, in0=wyv, in1=vm1v, op=Alu.mult)   # cy1
                V.tensor_scalar(out=tav, in0=wxv, scalar1=-1.0, scalar2=1.0,
                                op0=Alu.mult, op1=Alu.add)
                V.tensor_tensor(out=vc0v, in0=tav, in1=vc0v, op=Alu.mult)  # cx0
                V.tensor_tensor(out=vc1v, in0=wxv, in1=vc1v, op=Alu.mult)  # cx1

                wq5 = wq[ci][:].rearrange("p (t k l v) -> p t k l v", t=nt, k=K, l=2)
                V.tensor_tensor(out=wq5[:, :, :, 0, 0], in0=cAv, in1=vc0v, op=Alu.mult)
                V.tensor_tensor(out=wq5[:, :, :, 0, 1], in0=cBv, in1=vc0v, op=Alu.mult)
                V.tensor_tensor(out=wq5[:, :, :, 1, 0], in0=cAv, in1=vc1v, op=Alu.mult)
                V.tensor_tensor(out=wq5[:, :, :, 1, 1], in0=cBv, in1=vc1v, op=Alu.mult)

            # ---- remaining constant loads ----
            w2_sb = cpool.tile([128, 5 * Co], f32)
            nc.sync.dma_start(
                out=w2_sb[:].rearrange("p (f c) -> p f c", f=5),
                in_=w2_ap.rearrange("(f p) c -> p f c", p=128),
            )
            bias_sb = cpool.tile([Co, 1], f32)
            nc.sync.dma_start(out=bias_sb[:], in_=bias_ap)

            gpool = ctx.enter_context(tc.tile_pool(name="gather", bufs=12))
            wgpool = ctx.enter_context(tc.tile_pool(name="wg", bufs=4))
            spool = ctx.enter_context(tc.tile_pool(name="samp", bufs=4))
            stpool = ctx.enter_context(tc.tile_pool(name="sampT", bufs=4))
            obpool = ctx.enter_context(tc.tile_pool(name="ob", bufs=3))

            pads = (padT_ap, padB_ap)
            wqxpool = ctx.enter_context(tc.tile_pool(name="wqx", bufs=4))

            g_tiles = {}
            glast = ctx.enter_context(tc.tile_pool(name="glast", bufs=1))

            def chunk_of(t):
                for ci, (t0, t1, h) in enumerate(CHUNKS):
                    if t0 <= t < t1:
                        return ci, t - t0, h, t1 - t0
                raise AssertionError(t)

            def emit_gathers_last(t):
                # the final tile gathers into NINE separate tiles so each
                # tap's bilinear depends only on its own gather (Tile tracks
                # deps per tile, not per range)
                ci, tl, h, nt = chunk_of(t)
                idxi3 = idxi[ci][:].rearrange("p (t k) -> p t k", t=nt)
                gs = []
                for k in range(K):
                    gk = glast.tile([128, 4 * C], bf16, name=f"gl{k}",
                                    tag=f"gl{k}")
                    nc.gpsimd.indirect_dma_start(
                        out=gk[:], out_offset=None, in_=pads[h],
                        in_offset=bass.IndirectOffsetOnAxis(
                            ap=idxi3[:, tl, k:k + 1], axis=0),
                    )
                    gs.append(gk)
                g_tiles[t] = gs

            def emit_gathers(t):
                ci, tl, h, nt = chunk_of(t)
                idxi3 = idxi[ci][:].rearrange("p (t k) -> p t k", t=nt)
                g = gpool.tile([128, K * 4 * C], bf16, name=f"g{t}", tag="g")
                g_tiles[t] = g
                for k in range(K):
                    nc.gpsimd.indirect_dma_start(
                        out=g[:, k * 4 * C:(k + 1) * 4 * C],
                        out_offset=None,
                        in_=pads[h],
                        in_offset=bass.IndirectOffsetOnAxis(
                            ap=idxi3[:, tl, k:k + 1], axis=0),
                    )

            def emit_compute(t):
                ci, tl, h, nt = chunk_of(t)
                g = g_tiles.pop(t)
                # expand wq (k,l,v) -> (k,l,v,c) on the Scalar engine so the
                # DVE multiply below is fully contiguous bf16
                wq_t = wq[ci][:].rearrange(
                    "p (t k l v) -> p t k l v", t=nt, k=K, l=2)[:, tl]
                wq_b = wq_t.unsqueeze(4).to_broadcast([128, K, 2, 2, C])
                wqx = wqxpool.tile([128, K * 4 * C], bf16, name=f"wqx{t}",
                                   tag="wqx")
                nc.scalar.activation(
                    out=wqx[:].rearrange("p (k l v c) -> p k l v c",
                                         k=K, l=2, v=2),
                    in_=wq_b, func=Act.Copy)
                wg = wgpool.tile([128, K * 4 * C], bf16, name=f"wg{t}", tag="wg")
                V.tensor_tensor(out=wg[:], in0=g[:], in1=wqx[:], op=Alu.mult)

                wg5 = wg[:].rearrange("p (k l v c) -> p k l v c", k=K, l=2, v=2)
                s01 = spool.tile([128, K * 2 * C], bf16, tag="s01")
                s013 = s01[:].rearrange("p (k v c) -> p k v c", k=K, v=2)
                V.tensor_tensor(out=s013, in0=wg5[:, :, 0, :, :],
                                in1=wg5[:, :, 1, :, :], op=Alu.add)
                samp = spool.tile([128, K * C], bf16, tag="samp")
                samp3 = samp[:].rearrange("p (k c) -> p k c", k=K)
                V.tensor_tensor(out=samp3, in0=s013[:, :, 0, :],
                                in1=s013[:, :, 1, :], op=Alu.add)

                sampT = stpool.tile([128, 5 * 128], f32, name=f"sampT{t}",
                                    tag="sampT")
                for ci, off in enumerate(CHUNK_OFFS):
                    pt = tb_ps.tile([128, 128], bf16, tag="trB")
                    nc.tensor.transpose(out=pt[:], in_=samp[:, off:off + 128],
                                        identity=identb[:])
                    nc.scalar.activation(
                        out=sampT[:, ci * 128:(ci + 1) * 128], in_=pt[:],
                        func=Act.Copy)

                po = opsum.tile([Co, 128], f32, name=f"po{t}", tag="po")
                w2v = w2_sb[:].rearrange("p (f c) -> p f c", f=5)
                for ci in range(5):
                    nc.tensor.matmul(
                        out=po[:], lhsT=w2v[:, ci, :],
                        rhs=sampT[:, ci * 128:(ci + 1) * 128],
                        start=(ci == 0), stop=(ci == 4))

                ob = obpool.tile([Co, 128], f32, name=f"ob{t}", tag="ob")
                nc.scalar.activation(out=ob[:], in_=po[:], func=Act.Identity,
                                     bias=bias_sb[:, 0:1])
                nc.sync.dma_start(out=out_ap[:, t * 128:(t + 1) * 128], in_=ob[:])

            # ---- phase B: gathers lead the compute by a few tiles so the
            # gpsimd stream never waits on the vector/scalar pipeline ----
            def emit_compute_last(t):
                # per-tap pipeline for the final tile: each tap's bilinear
                # and each transpose chunk start as soon as their gathers
                # land, shortening the post-stream drain
                ci, tl, h, nt = chunk_of(t)
                gs = g_tiles.pop(t)
                wq_t = wq[ci][:].rearrange(
                    "p (t k l v) -> p t k l v", t=nt, k=K, l=2)[:, tl]
                wq_b = wq_t.unsqueeze(4).to_broadcast([128, K, 2, 2, C])
                wqx = wqxpool.tile([128, K * 4 * C], bf16, name=f"wqx{t}",
                                   tag="wqx")
                nc.scalar.activation(
                    out=wqx[:].rearrange("p (k l v c) -> p k l v c",
                                         k=K, l=2, v=2),
                    in_=wq_b, func=Act.Copy)
                wg = wgpool.tile([128, K * 4 * C], bf16, name=f"wg{t}", tag="wg")
                wg5 = wg[:].rearrange("p (k l v c) -> p k l v c", k=K, l=2, v=2)
                s01 = spool.tile([128, K * 2 * C], bf16, tag="s01")
                s013 = s01[:].rearrange("p (k v c) -> p k v c", k=K, v=2)
                # samp split (taps 0-3 | 4-5 | 6-8) so each transpose chunk
                # depends only on its own taps' reduction
                sA = glast.tile([128, 4 * C], bf16, name="sA", tag="sA")
                sB = glast.tile([128, 2 * C], bf16, name="sB", tag="sB")
                sC = glast.tile([128, 3 * C], bf16, name="sC", tag="sC")
                samp_out = [sA[:, k * C:(k + 1) * C] for k in range(4)] + \
                    [sB[:, (k - 4) * C:(k - 3) * C] for k in range(4, 6)] + \
                    [sC[:, (k - 6) * C:(k - 5) * C] for k in range(6, K)]
                tr_in = (sA[:, 0:128], sA[:, 128:256], sB[:, 0:128],
                         sC[:, 0:128], sC[:, 64:192])
                sampT = stpool.tile([128, 5 * 128], f32, name=f"sampT{t}",
                                    tag="sampT")
                po = opsum.tile([Co, 128], f32, name=f"po{t}", tag="po")
                w2v = w2_sb[:].rearrange("p (f c) -> p f c", f=5)
                tap_chunks = {1: (0,), 3: (1,), 5: (2,), 7: (3,), 8: (4,)}
                for k in range(K):
                    sl = slice(k * 4 * C, (k + 1) * 4 * C)
                    V.tensor_tensor(out=wg[:, sl], in0=gs[k][:],
                                    in1=wqx[:, sl], op=Alu.mult)
                    V.tensor_tensor(out=s013[:, k], in0=wg5[:, k, 0],
                                    in1=wg5[:, k, 1], op=Alu.add)
                    V.tensor_tensor(out=samp_out[k], in0=s013[:, k, 0],
                                    in1=s013[:, k, 1], op=Alu.add)
                    for c2 in tap_chunks.get(k, ()):
                        pt = tb_ps.tile([128, 128], bf16, tag="trB")
                        nc.tensor.transpose(out=pt[:],
                                            in_=tr_in[c2],
                                            identity=identb[:])
                        nc.scalar.activation(
                            out=sampT[:, c2 * 128:(c2 + 1) * 128], in_=pt[:],
                            func=Act.Copy)
                        nc.tensor.matmul(
                            out=po[:], lhsT=w2v[:, c2, :],
                            rhs=sampT[:, c2 * 128:(c2 + 1) * 128],
                            start=(c2 == 0), stop=(c2 == 4))
                ob = obpool.tile([Co, 128], f32, name=f"ob{t}", tag="ob")
                nc.scalar.activation(out=ob[:], in_=po[:], func=Act.Identity,
                                     bias=bias_sb[:, 0:1])
                nc.sync.dma_start(out=out_ap[:, t * 128:(t + 1) * 128], in_=ob[:])

            LEAD = 10
            a3a(0)
            emit_gathers(0)
            a3a(1)
            for t in range(1, 4):
                emit_gathers(t)
            for ci in range(2, len(CHUNKS)):
                a3a(ci)
            for t in range(4, LEAD):
                emit_gathers(t)
            for ci in range(len(CHUNKS)):
                a3b(ci)
            for t in range(NT):
                if t + LEAD < NT:
                    if t + LEAD == NT - 1:
                        emit_gathers_last(t + LEAD)
                    else:
                        emit_gathers(t + LEAD)
                if t == NT - 1:
                    emit_compute_last(t)
                else:
                    emit_compute(t)

    nc.compile()
    nc.m = get_hw_module(nc.m)
    return nc


def _host_prep(input, offset, mask, weight, bias):
    import ml_dtypes

    f32 = np.float32
    bf16 = ml_dtypes.bfloat16
    input = np.ascontiguousarray(input, dtype=f32)
    offset = np.ascontiguousarray(offset, dtype=f32)
    mask = np.ascontiguousarray(mask, dtype=f32)
    weight = np.ascontiguousarray(weight, dtype=f32)
    bias = np.ascontiguousarray(bias, dtype=f32)

    # The split-pad dependency scheme requires sample rows to stay within
    # each half's tensor range; |offset| < 6 gives margin of >900 rows.
    amax = float(np.abs(offset).max())
    assert amax < 6.0, f"offset magnitude {amax} exceeds pad-split safety bound"

    # weight [Co, C, 3, 3] -> W2r[(t*64+c), co], chunked at CHUNK_OFFS with
    # the 448-overlap region zeroed out of chunk 4 (rows 448..511 live in
    # chunk 3).
    wr = weight.reshape(Co, C, K)                     # [co, c, t]
    W2r = np.transpose(wr, (2, 1, 0)).reshape(C * K, Co)  # [(t,c), co]
    w2 = np.zeros((5, 128, Co), dtype=f32)
    w2[0] = W2r[0:128]
    w2[1] = W2r[128:256]
    w2[2] = W2r[256:384]
    w2[3] = W2r[384:512]
    w2[4, 64:128] = W2r[512:576]
    w2 = w2.reshape(5 * 128, Co)

    biasv = bias.reshape(Co, 1)
    kyv = (np.arange(K, dtype=f32) // 3)
    kxv = (np.arange(K, dtype=f32) % 3)

    pix = np.arange(NPIX).reshape(NT, 128)
    in_maps = []
    for core in range(N_CORES):
        b, h = core // 2, core % 2
        ho0 = h * HHALF
        ho = ho0 + pix // W
        wo = pix % W
        base_y = (ho - 1)[:, :, None] + kyv[None, None, :]   # [NT, 128, K]
        base_x = (wo - 1)[:, :, None] + kxv[None, None, :]
        byx = np.stack([base_y, base_x], axis=-1)            # [NT, 128, K, 2]
        byx = np.ascontiguousarray(
            byx.transpose(1, 0, 2, 3).reshape(128, NT * K * 2), dtype=f32)
        # offset/mask, pixel-major: omT[p, t*27+j] = om[j, t*128+p]
        om = np.concatenate(
            [offset[b, :, ho0:ho0 + HHALF, :].reshape(18, NPIX),
             mask[b, :, ho0:ho0 + HHALF, :].reshape(K, NPIX)], axis=0)
        omT = np.ascontiguousarray(
            om.reshape(27, NT, 128).transpose(2, 1, 0).reshape(128, NT * 27))
        # quad-packed bf16 pads: pad[r] = [pix(base+r-97) | pix(base+r-1)],
        # zero outside the image.  Global bases per (core-half, tile-half):
        # half A covers output rows [48h, 48h+24), half B [48h+24, 48h+48).
        P = np.ascontiguousarray(input[b].reshape(C, HW).T).astype(bf16)
        bases = (0, 1536) if h == 0 else (3936, 6336)

        def build_pad(base):
            pad = np.zeros((PAD_ROWS, 2 * C), dtype=bf16)
            for col, shift in ((0, 97), (C, 1)):
                p0 = base - shift            # pixel at local row 0
                lo = max(0, -p0)             # first local row with a pixel
                hi = min(PAD_ROWS, HW - p0)  # one past last local row
                if hi > lo:
                    pad[lo:hi, col:col + C] = P[p0 + lo:p0 + hi]
            return pad

        basev = np.tile(np.array([[97 - bases[0], 97 - bases[1]]],
                                 dtype=f32), (128, 1))
        in_maps.append({
            "omT": omT,
            "byx": byx,
            "w2": w2,
            "biasv": biasv,
            "padtop": build_pad(bases[0]),
            "padbot": build_pad(bases[1]),
            "basev": basev,
        })
    return in_maps


def kernel(input, offset, mask, weight, bias):
    from concourse.bass_utils import run_bass_kernel_spmd

    if "nc" not in _CACHE:
        _CACHE["nc"] = _build_module()
    nc = _CACHE["nc"]

    in_maps = _host_prep(input, offset, mask, weight, bias)
    res = run_bass_kernel_spmd(nc, in_maps, core_ids=list(range(N_CORES)))

    out = np.empty((B, Co, H, W), dtype=np.float32)
    for core in range(N_CORES):
        b, h = core // 2, core % 2
        ho0 = h * HHALF
        out[b, :, ho0:ho0 + HHALF, :] = \
            res.results[core]["out"].reshape(Co, HHALF, W)
    return out

